# revision 6
# baseline (speedup 1.0000x reference)
"""Trainium2 Bass kernel for nn_PermutedSparseWeight.

Math: out = P0-mix( P1-mix( X*mask ) ) where both mixes are weighted sums
over 8 block-local (64-wide) permutations.  Because every permutation maps
indices within their own 64-block, the whole computation factors into
per-block matrix sandwiches:

    out[block a, block b] = B_a @ (X*mask)[a, b] @ A_b

with B_a[j, m] = sum_p c0[p, j]·[perm0[p, j] == m]   (row mix)
and  A_b[c, k] = sum_p c1[p, k]·[perm1[p, k] == c]   (col mix).

The tiny A/B matrices (1 MB each) are assembled on the host from the
c/perm metadata; all heavy data (X 64 MB, mask 16 MB, out 64 MB) is
processed on device.  d_out is sharded 8 ways (512 rows / core, a
multiple of the 64 block size, keeping row mixes core-local); A is
replicated.

On device, per 128-row chunk (2 blocks): an fp16 matmul with the masked
X-chunk as the stationary operand produces the row-mixed chunk directly
in transposed layout (out1T[c, j] = sum_m Wm[m, c]·BT[m, j]), which is
exactly the lhsT layout the column-mix matmul needs — no transposes.
The mask ships as fp8 (float dtype keeps the DVE multiply on its fast
mixed-float path); all 16-bit dtypes stay well inside the 2e-2 gate
(measured rel err 1.4e-3).
"""

import numpy as np

D = 4096
NP = 8
BLOCK = 64
NCORES = 8
P = 128

_CACHE = {}
PROFILE = False  # test-harness switch: capture NTFF profile on the next run
LAST = {}  # test-harness: BassKernelResults of the most recent run
# matmul dtypes, output dtype and mask handling; see build_bass
CONFIG = {"mm1": "fp16", "mm2": "fp16", "mask": "fp8", "out": "fp16", "x": "fp16"}


_MAXW = 1  # walrus ISA: instructions carry at most one sync wait command


def _patch_tile_drain():
    """The walrus codegen in this environment rejects instructions carrying
    more than _MAXW semaphore waits ("Too many sync wait commands").  Two
    patches, both semantically neutral:
      1. every instruction Tile commits with more waits gets same-engine
         no-op predecessors carrying the overflow waits (engine queues are
         in-order, so the waits still all complete before the instruction);
      2. the TileContext exit drain is split into a chain of drains."""
    import concourse.tile as tile
    import bass_rust
    from concourse.vector_clock import ScopedClock

    if getattr(tile.TileContext, "_drain_patched", False):
        return

    def _split_waits(self, inst):
        si = inst.sync_info
        waits = list(si.on_wait or []) if si else []
        if len(waits) <= _MAXW:
            return
        keep = waits[-_MAXW:]
        extra = waits[: -_MAXW]
        for i in range(0, len(extra), _MAXW):
            nop = bass_rust.InstNoOp(name=self.nc.get_next_instruction_name())
            nop.engine = inst.engine
            nop.sync_info = bass_rust.SyncInfo(
                on_wait=extra[i : i + _MAXW], on_update=[]
            )
            self.nc.register_instruction(nop, overwrite=True)
            self.nc.cur_bb.bb.add_instruction(nop)
        inst.sync_info = bass_rust.SyncInfo(
            on_wait=keep, on_update=list(si.on_update or [])
        )

    orig_add = tile.TileContext._add_instruction

    def _add_instruction(self, inst):
        if inst.engine != tile.mybir.EngineType.Unassigned:
            _split_waits(self, inst)
        orig_add(self, inst)

    def _drain_and_barrier(self, tick_clock, wait_clock):
        drain_inst = self.nc.sync.drain()
        wait_clock.add_sem_waits(
            drain_inst.ins, ScopedClock({None: tick_clock.global_clock})
        )
        si = drain_inst.ins.sync_info
        waits = list(si.on_wait or []) if si else []
        if len(waits) > _MAXW:
            drain_inst.ins.sync_info = bass_rust.SyncInfo(
                on_wait=waits[:_MAXW], on_update=list(si.on_update or [])
            )
            for i in range(_MAXW, len(waits), _MAXW):
                d2 = self.nc.sync.drain()
                si2 = d2.ins.sync_info
                upd = list(si2.on_update or []) if si2 else []
                d2.ins.sync_info = bass_rust.SyncInfo(
                    on_wait=waits[i : i + _MAXW], on_update=upd
                )
        self.nc.all_engine_barrier()
        assert self.sems is not None
        popped = self.nc._tile_sem_poison_stack.pop()
        assert popped is self._sem_poison
        self.nc.clear_and_free_semaphores(list(self.sems.allocated().values()))
        self.nc.all_engine_barrier()

    tile.TileContext._add_instruction = _add_instruction
    tile.TileContext._drain_and_barrier = _drain_and_barrier
    tile.TileContext._drain_patched = True


def build_bass(rows, d, mm1="fp16", mm2="fp16", mask="fp16", out="fp16", x="fp16"):
    """One-core SPMD program: rows x d shard of X/mask -> rows x d of out.

    mm1/mm2: dtype of the row-mix / col-mix matmuls ("f32"/"bf16"/"fp16").
    out: dtype of the output DRAM tensor (16-bit halves the store traffic;
    the host upcasts back to f32).
    x: dtype X ships in.  fp16 halves the X DMA (the dominant load) and is
    precision-neutral: w_t is already fp16, so quantizing X before the mask
    multiply gives the identical product.  The fp16*fp8 multiply runs the
    same 1x DVE path as the old f32*fp8 (2x needs ALL operands 2-byte).

    DMA-packet economics (measured): per-packet cost ~110ns (2KB) /
    ~165ns (4KB) / ~310ns (8KB) across 16 engines => >=4KB rows sustain
    ~400 GB/s aggregate, 2KB rows ~300.  So steady-state pieces keep
    >=4KB DRAM rows: X fp16 2048-wide, masks fp8 full-width 4096, amat
    fp16 full-width, stores fp16 2048-wide.

    Ring plan: sync HWDGE carries all X (first, in consumption order) then
    the last chunk's stores; scalar HWDGE carries m0 (split halves for an
    early first multiply), bt, amat; gpsimd SWDGE carries m1-m3 and the
    other chunks' stores.  PSUM evictions are split across
    vector/scalar/gpsimd so no single engine becomes the drain."""
    import concourse.bass as bass
    import concourse.tile as tile
    from concourse import mybir

    _patch_tile_drain()

    f32 = mybir.dt.float32
    bf16 = mybir.dt.bfloat16
    fp16 = mybir.dt.float16
    u8 = mybir.dt.uint8
    dmap = {"f32": f32, "bf16": bf16, "fp16": fp16, "fp8": mybir.dt.float8e4}
    mm1_dt = dmap[mm1]
    mm2_dt = dmap[mm2]
    out_dt = dmap[out]
    m_dt = u8 if mask == "u8" else dmap[mask]
    x_dt = dmap[x]

    rc_n = rows // P      # row chunks per core (4)
    cch = d // P          # column chunks (32)
    grp = 4               # col chunks per PSUM bank group / 512-col strip
    gw = grp * P          # 512
    qw = 2 * gw           # pair width (1024)

    nc = bass.Bass("TRN2", target_bir_lowering=False, debug=False)
    # NOTE: nc.m.queues each declare num_queues=16 subqueue rings; transfers
    # round-robin across the rings to feed the 16 DMA engines.  Shrinking the
    # declaration (to cut the NEFF epilogue's per-subqueue semaphore clears,
    # ~6 us) collapses DMA throughput ~3.5x — do not touch it.
    x_d = nc.dram_tensor("x", [rows, d], x_dt, kind="ExternalInput").ap()
    m_d = nc.dram_tensor("m", [rows, d], m_dt, kind="ExternalInput").ap()
    bt_d = nc.dram_tensor("bt", [P, rc_n * P], mm1_dt, kind="ExternalInput").ap()
    a_d = nc.dram_tensor("amat", [P, d], mm2_dt, kind="ExternalInput").ap()
    o_d = nc.dram_tensor("out", [rows, d], out_dt, kind="ExternalOutput").ap()

    with tile.TileContext(nc) as tc:
        with (
            tc.tile_pool(name="const", bufs=1) as constp,
            tc.tile_pool(name="xin", bufs=8) as xp,
            tc.tile_pool(name="min", bufs=4) as mp,
            tc.tile_pool(name="wq", bufs=3) as wp,
            tc.tile_pool(name="o1", bufs=3) as o1p,
            tc.tile_pool(name="osb", bufs=2) as outp,
            tc.tile_pool(name="ps1", bufs=2, space="PSUM") as ps1p,
            tc.tile_pool(name="ps2", bufs=2, space="PSUM") as ps2p,
        ):
            # ---- prefetch: every load issued before any compute ----
            # Warm the ACT table during the DMA ramp: the first real
            # ACTIVATE otherwise pays a ~1.3us ACT_TABLE_LOAD on the
            # chunk-0 critical path.
            warm = constp.tile([P, 2], f32, name="actwarm")
            nc.gpsimd.memset(warm[:, 0:1], 0.0)
            nc.scalar.copy(warm[:, 1:2], warm[:, 0:1])

            # sync ring: X pieces in consumption order (chunk 0 split
            # 4x1024 so the first multiply starts early), with each later
            # chunk's mask interleaved after its X so m_c lands just after
            # x_c.
            xw = 2 * qw  # steady-state X DMA width (2048)
            xs = [None] * rc_n
            m_ts = [None] * rc_n
            for rc in range(rc_n):
                fast = rc == 0
                pw = qw if fast else xw
                ps_ = []
                for j in range(d // pw):
                    x_t = xp.tile(
                        [P, pw], x_dt, name=f"x{rc}_{j}", tag=f"x{int(fast)}_t"
                    )
                    nc.sync.dma_start(
                        x_t[:], x_d[rc * P : (rc + 1) * P, j * pw : (j + 1) * pw]
                    )
                    ps_.append(x_t)
                xs[rc] = (pw, ps_)
                if rc > 0:
                    m_t = mp.tile([P, d], m_dt, name=f"m{rc}", tag="m_t")
                    nc.sync.dma_start(m_t[:], m_d[rc * P : (rc + 1) * P, :])
                    m_ts[rc] = m_t

            # scalar ring: m0 first half (first multiply dep), bt (first mm1
            # dep), m0 second half, amat (single full-width 8KB-row DMA,
            # first mm2 dep), in need order.
            m0 = mp.tile([P, d], m_dt, name="m0", tag="m_t")
            h = d // 2
            nc.scalar.dma_start(m0[:, 0:h], m_d[0:P, 0:h])
            bt_t = constp.tile([P, rc_n * P], mm1_dt)
            nc.scalar.dma_start(bt_t[:], bt_d[:])
            nc.scalar.dma_start(m0[:, h:d], m_d[0:P, h:d])
            m_ts[0] = m0
            a_t = constp.tile([P, d], mm2_dt, name="amat", tag="amat")
            nc.scalar.dma_start(a_t[:], a_d[:])

            # ---- compute ----
            # Per chunk: mask-multiplies (vector 3/4, gpsimd 1/4), then 4
            # "pairs" of 1024 cols: 8 mm1 matmuls into a 2-bank PSUM tile,
            # a 1024-wide o1 eviction, 8 mm2 matmuls, a 1024-wide eviction
            # into the store buffer.  Stores: chunks 0-2 go 2048-wide on
            # gpsimd SWDGE; the last chunk's go on the sync HWDGE ring
            # (idle once loads finish) with fine-grained evictions to cut
            # the drain latency.
            # Engine budget (measured rates ~1.2ns/col DVE, ~0.9 ACT, ~1.4
            # Pool; only DVE/ACT can read PSUM): gpsimd takes nearly all
            # mask-multiplies, scalar takes every o1 eviction plus a quarter
            # of the out evictions, vector takes chunk-0/last-piece
            # multiplies plus the remaining out evictions.  Triggers live on
            # sync (idle) and gpsimd.
            sw = 2 * qw  # store piece width (2048)
            for rc in range(rc_n):
                rs = slice(rc * P, (rc + 1) * P)
                m_t = m_ts[rc]
                oh = [
                    outp.tile([P, sw], out_dt, name=f"oq{q}", tag=f"oq{q}")
                    for q in range(2)
                ]
                pw, xpieces = xs[rc]
                last = rc == rc_n - 1
                fine = last  # fine-grained evictions only in the final chunk
                wref = []  # per 1024-col pair: (w tile, col offset)
                if pw == qw:
                    for j in range(4):
                        w_t = wp.tile([P, qw], mm1_dt, name="w0_t", tag="w0_t")
                        meng = nc.vector if j < 3 else nc.gpsimd
                        meng.tensor_mul(
                            w_t[:],
                            xpieces[j][:],
                            m_t[:, j * qw : (j + 1) * qw],
                        )
                        wref.append((w_t, 0))
                else:
                    w0 = wp.tile([P, xw], mm1_dt, name="w_t", tag="w_t")
                    nc.gpsimd.tensor_mul(
                        w0[:], xpieces[0][:], m_t[:, 0:xw]
                    )
                    w1 = wp.tile([P, xw], mm1_dt, name="w_t2", tag="w_t")
                    if last:
                        # split the final multiply across engines: shortest
                        # path from the last X piece to the last matmul
                        nc.vector.tensor_mul(
                            w1[:, 0:qw], xpieces[1][:, 0:qw], m_t[:, xw : xw + qw]
                        )
                        nc.gpsimd.tensor_mul(
                            w1[:, qw:xw], xpieces[1][:, qw:xw], m_t[:, xw + qw : 2 * xw]
                        )
                    else:
                        nc.gpsimd.tensor_mul(
                            w1[:], xpieces[1][:], m_t[:, xw : 2 * xw]
                        )
                    wref = [(w0, 0), (w0, qw), (w1, 0), (w1, qw)]
                for p in range(4):
                    ps1 = ps1p.tile([P, qw], f32)
                    w_t, wo = wref[p]
                    for t in range(2 * grp):
                        nc.tensor.matmul(
                            ps1[:, t * P : (t + 1) * P],
                            lhsT=w_t[:, wo + t * P : wo + (t + 1) * P],
                            rhs=bt_t[:, rc * P : (rc + 1) * P],
                            start=True,
                            stop=True,
                        )
                    o1 = o1p.tile([P, qw], mm2_dt)
                    if fine:
                        # 512-wide halves on two engines: halves the
                        # mm1->mm2 latency in the drain chunks
                        nc.scalar.copy(o1[:, 0:gw], ps1[:, 0:gw])
                        nc.vector.tensor_copy(o1[:, gw:qw], ps1[:, gw:qw])
                    else:
                        nc.scalar.copy(o1[:], ps1[:])
                    ps2 = ps2p.tile([P, qw], f32)
                    for t in range(2 * grp):
                        c = p * 2 * grp + t
                        nc.tensor.matmul(
                            ps2[:, t * P : (t + 1) * P],
                            lhsT=o1[:, t * P : (t + 1) * P],
                            rhs=a_t[:, c * P : (c + 1) * P],
                            start=True,
                            stop=True,
                        )
                    j = p // 2
                    off = (p % 2) * qw
                    if fine:
                        nc.scalar.copy(oh[j][:, off : off + gw], ps2[:, 0:gw])
                        nc.vector.tensor_copy(
                            oh[j][:, off + gw : off + qw], ps2[:, gw:qw]
                        )
                    elif p == 3:
                        nc.scalar.copy(oh[j][:, off : off + qw], ps2[:])
                    else:
                        nc.vector.tensor_copy(oh[j][:, off : off + qw], ps2[:])
                    if p % 2 == 1:
                        seng = nc.gpsimd if rc in (0, 2) else nc.sync
                        seng.dma_start(o_d[rs, j * sw : (j + 1) * sw], oh[j][:])
    return nc


def host_prep(c_0, c_1, permutations_0, permutations_1, d):
    """Build the block-diagonal mix matrices.

    Returns bt_all [d//128, 128, 128] (chunk, m_local, j_local) and
    amat [128, d] (c_local, chunk*128 + k_local)."""
    k = np.arange(d)
    p0 = np.asarray(permutations_0)
    p1 = np.asarray(permutations_1)
    c0 = np.asarray(c_0, dtype=np.float32)
    c1 = np.asarray(c_1, dtype=np.float32)
    cch = d // P

    bt = np.zeros((d, BLOCK), np.float32)  # [j, m_local]
    for p in range(p0.shape[0]):
        np.add.at(bt, (k, p0[p] % BLOCK), c0[p])
    b4 = bt.reshape(cch, 2, BLOCK, BLOCK)  # [chunk, half, j_loc, m_loc]
    bt_all = np.zeros((cch, P, P), np.float32)
    bt_all[:, :BLOCK, :BLOCK] = b4[:, 0].transpose(0, 2, 1)
    bt_all[:, BLOCK:, BLOCK:] = b4[:, 1].transpose(0, 2, 1)

    a1 = np.zeros((d, BLOCK), np.float32)  # [k, c_local]
    for p in range(p1.shape[0]):
        np.add.at(a1, (k, p1[p] % BLOCK), c1[p])
    a4 = a1.reshape(cch, 2, BLOCK, BLOCK)  # [chunk, half, k_loc, c_loc]
    a_all = np.zeros((cch, P, P), np.float32)
    a_all[:, :BLOCK, :BLOCK] = a4[:, 0].transpose(0, 2, 1)
    a_all[:, BLOCK:, BLOCK:] = a4[:, 1].transpose(0, 2, 1)
    amat = np.ascontiguousarray(a_all.transpose(1, 0, 2).reshape(P, d))
    return bt_all, amat


def _numpy_fallback(X, c_0, c_1, mask, p0, p1):
    W = np.asarray(X, np.float32) * np.asarray(mask)
    W = np.einsum("ipk,pk->ik", W[:, p1], np.asarray(c_1, np.float32))
    W = np.einsum("pjk,pj->jk", W[p0, :], np.asarray(c_0, np.float32))
    return W.astype(np.float32)


def kernel(X, c_0, c_1, mask, permutations_0, permutations_1):
    X = np.asarray(X)
    mask = np.asarray(mask)
    p0 = np.asarray(permutations_0)
    p1 = np.asarray(permutations_1)

    d = X.shape[1]
    k = np.arange(d)
    block_local = (
        X.shape == (D, D)
        and p0.shape == (NP, D)
        and p1.shape == (NP, D)
        and (p0 // BLOCK == k // BLOCK).all()
        and (p1 // BLOCK == k // BLOCK).all()
    )
    if not block_local:
        return _numpy_fallback(X, c_0, c_1, mask, p0, p1)

    from concourse.bass_utils import run_bass_kernel_spmd

    rows = D // NCORES
    cfg = dict(CONFIG)
    key = ("nc", cfg["mm1"], cfg["mm2"], cfg["mask"], cfg["out"], cfg["x"])
    if key not in _CACHE:
        _CACHE[key] = build_bass(rows, D, **cfg)
    nc = _CACHE[key]

    def _mmdt(which):
        if cfg[which] == "bf16":
            import ml_dtypes

            return ml_dtypes.bfloat16
        if cfg[which] == "fp16":
            return np.float16
        if cfg[which] == "fp8":
            import ml_dtypes

            return ml_dtypes.float8_e4m3fn
        return np.float32

    bt_all, amat = host_prep(c_0, c_1, p0, p1, D)
    amat = np.ascontiguousarray(amat.astype(_mmdt("mm2")))
    rc_n = rows // P
    xf = np.ascontiguousarray(X.astype(_mmdt("x"), copy=False))
    mu = np.ascontiguousarray(mask.astype(_mmdt("mask") if cfg["mask"] != "u8" else np.uint8))

    in_maps = []
    for i in range(NCORES):
        rs = slice(i * rows, (i + 1) * rows)
        bt_core = np.ascontiguousarray(
            bt_all[i * rc_n : (i + 1) * rc_n]
            .transpose(1, 0, 2)
            .reshape(P, rc_n * P)
            .astype(_mmdt("mm1"))
        )
        in_maps.append(
            {
                "x": xf[rs],
                "m": mu[rs],
                "bt": bt_core,
                "amat": amat,
            }
        )

    res = run_bass_kernel_spmd(nc, in_maps, list(range(NCORES)), trace=PROFILE)
    LAST["res"] = res
    out = np.concatenate([res.results[i]["out"] for i in range(NCORES)], axis=0)
    return out.astype(np.float32)



# revision 8
# speedup vs baseline: 1.0441x; 1.0441x over previous
"""Trainium2 Bass kernel for nn_PermutedSparseWeight.

Math: out = P0-mix( P1-mix( X*mask ) ) where both mixes are weighted sums
over 8 block-local (64-wide) permutations.  Because every permutation maps
indices within their own 64-block, the whole computation factors into
per-block matrix sandwiches:

    out[block a, block b] = B_a @ (X*mask)[a, b] @ A_b

with B_a[j, m] = sum_p c0[p, j]·[perm0[p, j] == m]   (row mix)
and  A_b[c, k] = sum_p c1[p, k]·[perm1[p, k] == c]   (col mix).

The tiny A/B matrices (1 MB each) are assembled on the host from the
c/perm metadata; all heavy data (X 64 MB, mask 16 MB, out 64 MB) is
processed on device.  d_out is sharded 8 ways (512 rows / core, a
multiple of the 64 block size, keeping row mixes core-local); A is
replicated.

On device, per 128-row chunk (2 blocks): an fp16 matmul with the masked
X-chunk as the stationary operand produces the row-mixed chunk directly
in transposed layout (out1T[c, j] = sum_m Wm[m, c]·BT[m, j]), which is
exactly the lhsT layout the column-mix matmul needs — no transposes.
The mask ships as fp8 (float dtype keeps the DVE multiply on its fast
mixed-float path); all 16-bit dtypes stay well inside the 2e-2 gate
(measured rel err 1.4e-3).
"""

import numpy as np

D = 4096
NP = 8
BLOCK = 64
NCORES = 8
P = 128

_CACHE = {}
PROFILE = False  # test-harness switch: capture NTFF profile on the next run
LAST = {}  # test-harness: BassKernelResults of the most recent run
# matmul dtypes, output dtype and mask handling; see build_bass
CONFIG = {"mm1": "fp16", "mm2": "fp16", "mask": "fp16", "out": "fp16", "x": "fp16"}


_MAXW = 1  # walrus ISA: instructions carry at most one sync wait command


def _patch_tile_drain():
    """The walrus codegen in this environment rejects instructions carrying
    more than _MAXW semaphore waits ("Too many sync wait commands").  Two
    patches, both semantically neutral:
      1. every instruction Tile commits with more waits gets same-engine
         no-op predecessors carrying the overflow waits (engine queues are
         in-order, so the waits still all complete before the instruction);
      2. the TileContext exit drain is split into a chain of drains."""
    import concourse.tile as tile
    import bass_rust
    from concourse.vector_clock import ScopedClock

    if getattr(tile.TileContext, "_drain_patched", False):
        return

    def _split_waits(self, inst):
        si = inst.sync_info
        waits = list(si.on_wait or []) if si else []
        if len(waits) <= _MAXW:
            return
        keep = waits[-_MAXW:]
        extra = waits[: -_MAXW]
        for i in range(0, len(extra), _MAXW):
            nop = bass_rust.InstNoOp(name=self.nc.get_next_instruction_name())
            nop.engine = inst.engine
            nop.sync_info = bass_rust.SyncInfo(
                on_wait=extra[i : i + _MAXW], on_update=[]
            )
            self.nc.register_instruction(nop, overwrite=True)
            self.nc.cur_bb.bb.add_instruction(nop)
        inst.sync_info = bass_rust.SyncInfo(
            on_wait=keep, on_update=list(si.on_update or [])
        )

    orig_add = tile.TileContext._add_instruction

    def _add_instruction(self, inst):
        if inst.engine != tile.mybir.EngineType.Unassigned:
            _split_waits(self, inst)
        orig_add(self, inst)

    def _drain_and_barrier(self, tick_clock, wait_clock):
        drain_inst = self.nc.sync.drain()
        wait_clock.add_sem_waits(
            drain_inst.ins, ScopedClock({None: tick_clock.global_clock})
        )
        si = drain_inst.ins.sync_info
        waits = list(si.on_wait or []) if si else []
        if len(waits) > _MAXW:
            drain_inst.ins.sync_info = bass_rust.SyncInfo(
                on_wait=waits[:_MAXW], on_update=list(si.on_update or [])
            )
            for i in range(_MAXW, len(waits), _MAXW):
                d2 = self.nc.sync.drain()
                si2 = d2.ins.sync_info
                upd = list(si2.on_update or []) if si2 else []
                d2.ins.sync_info = bass_rust.SyncInfo(
                    on_wait=waits[i : i + _MAXW], on_update=upd
                )
        self.nc.all_engine_barrier()
        assert self.sems is not None
        popped = self.nc._tile_sem_poison_stack.pop()
        assert popped is self._sem_poison
        self.nc.clear_and_free_semaphores(list(self.sems.allocated().values()))
        self.nc.all_engine_barrier()

    tile.TileContext._add_instruction = _add_instruction
    tile.TileContext._drain_and_barrier = _drain_and_barrier
    tile.TileContext._drain_patched = True


def build_bass(rows, d, mm1="fp16", mm2="fp16", mask="fp16", out="fp16", x="fp16"):
    """One-core SPMD program: rows x d shard of X/mask -> rows x d of out.

    Dtype strategy (all HW-measured): the DVE/gpsimd multiply is fast
    (~0.6-1.2 ns/col) only for f32*fp8 and f16*f16 operand pairs; f16*fp8
    runs ~4.2 ns/col and bf16*bf16 ~2.4 ns/col.  Shipping BOTH X and mask
    as fp16 costs 2 MB/core more mask DMA than fp8 but halves the X DMA
    vs f32 (net -2 MB) and keeps the multiply on the fast path - and on
    gpsimd, so the two PSUM-capable engines (vector/scalar) spend their
    time exclusively on PSUM drains.  Precision is identical to the f32
    path: w is fp16 either way.  (In-DMA masking via accum_op is a dead
    end: DGE compute ops are bypass/add only.)

    DMA-packet economics (measured): >=4KB DRAM rows sustain ~400 GB/s
    aggregate; 2KB rows ~300.  X fp16 2048-wide pieces = 4KB rows; fp16
    masks and amat full-width = 8KB rows; stores 2048-wide = 4KB rows.

    Ring plan: sync HWDGE carries all X then stores c1/c3; scalar HWDGE
    carries m0 (split halves for an early first multiply), bt, amat, then
    m1-m3; gpsimd SWDGE carries stores c0/c2.  PSUM evictions split
    ~50/50 vector/scalar; all mask-multiplies on gpsimd (chunk 3's last
    piece splits with vector to shorten the drain tail)."""
    import concourse.bass as bass
    import concourse.tile as tile
    from concourse import mybir

    _patch_tile_drain()

    f32 = mybir.dt.float32
    fp16 = mybir.dt.float16
    u8 = mybir.dt.uint8
    dmap = {"f32": mybir.dt.float32, "bf16": mybir.dt.bfloat16,
            "fp16": fp16, "fp8": mybir.dt.float8e4}
    mm1_dt = dmap[mm1]
    mm2_dt = dmap[mm2]
    out_dt = dmap[out]
    m_dt = u8 if mask == "u8" else dmap[mask]
    x_dt = dmap[x]

    rc_n = rows // P      # row chunks per core (4)
    cch = d // P          # column chunks (32)
    grp = 4               # col chunks per PSUM bank group / 512-col strip
    gw = grp * P          # 512
    qw = 2 * gw           # pair width (1024)
    xw = 2 * qw           # x piece width (2048)

    nc = bass.Bass("TRN2", target_bir_lowering=False, debug=False)
    # NOTE: nc.m.queues each declare num_queues=16 subqueue rings; transfers
    # round-robin across the rings to feed the 16 DMA engines.  Shrinking the
    # declaration (to cut the NEFF epilogue's per-subqueue semaphore clears,
    # ~6 us) collapses DMA throughput ~3.5x - do not touch it.
    x_d = nc.dram_tensor("x", [rows, d], x_dt, kind="ExternalInput").ap()
    m_d = nc.dram_tensor("m", [rows, d], m_dt, kind="ExternalInput").ap()
    bt_d = nc.dram_tensor("bt", [P, rc_n * P], mm1_dt, kind="ExternalInput").ap()
    a_d = nc.dram_tensor("amat", [P, d], mm2_dt, kind="ExternalInput").ap()
    o_d = nc.dram_tensor("out", [rows, d], out_dt, kind="ExternalOutput").ap()

    with tile.TileContext(nc) as tc:
        with (
            tc.tile_pool(name="const", bufs=1) as constp,
            tc.tile_pool(name="xin", bufs=10) as xp,
            tc.tile_pool(name="min", bufs=4) as mp,
            tc.tile_pool(name="wq", bufs=4) as wp,
            tc.tile_pool(name="o1", bufs=3) as o1p,
            tc.tile_pool(name="osb", bufs=4) as outp,
            tc.tile_pool(name="ps1", bufs=2, space="PSUM") as ps1p,
            tc.tile_pool(name="ps2", bufs=2, space="PSUM") as ps2p,
        ):
            # Warm the ACT table during the DMA ramp: the first real
            # ACTIVATE otherwise pays a ~1.3us ACT_TABLE_LOAD on the
            # chunk-0 critical path.
            warm = constp.tile([P, 2], f32, name="actwarm")
            nc.gpsimd.memset(warm[:, 0:1], 0.0)
            nc.scalar.copy(warm[:, 1:2], warm[:, 0:1])

            # sync ring: all X pieces in consumption order.
            xs = [None] * rc_n
            for rc in range(rc_n):
                ps_ = []
                for j in range(d // xw):
                    x_t = xp.tile([P, xw], x_dt, name=f"x{rc}_{j}", tag="x_t")
                    nc.sync.dma_start(
                        x_t[:], x_d[rc * P : (rc + 1) * P, j * xw : (j + 1) * xw]
                    )
                    ps_.append(x_t)
                xs[rc] = ps_

            # scalar ring in need order: m0 first half, bt, m0 second half,
            # amat, then the remaining masks (fp16 full-width = 8KB rows).
            m_ts = [None] * rc_n
            m0 = mp.tile([P, d], m_dt, name="m0", tag="m_t")
            h = d // 2
            nc.scalar.dma_start(m0[:, 0:h], m_d[0:P, 0:h])
            bt_t = constp.tile([P, rc_n * P], mm1_dt)
            nc.scalar.dma_start(bt_t[:], bt_d[:])
            nc.scalar.dma_start(m0[:, h:d], m_d[0:P, h:d])
            m_ts[0] = m0
            a_t = constp.tile([P, d], mm2_dt, name="amat", tag="amat")
            nc.scalar.dma_start(a_t[:], a_d[:])
            for rc in range(1, rc_n):
                m_t = mp.tile([P, d], m_dt, name=f"m{rc}", tag="m_t")
                nc.scalar.dma_start(m_t[:], m_d[rc * P : (rc + 1) * P, :])
                m_ts[rc] = m_t

            # ---- compute ----
            # Per chunk: two 2048-wide fp16 mask-multiplies on gpsimd, then
            # 4 "pairs" of 1024 cols: 8 mm1 matmuls into a 2-bank PSUM
            # tile, a 1024-wide o1 eviction, 8 mm2 matmuls, a 1024-wide
            # eviction into the store buffer.  Evictions alternate
            # scalar/vector; stores go 2048-wide on gpsimd (c0/c2) and
            # sync (c1/c3).
            sw = 2 * qw  # store piece width (2048)
            for rc in range(rc_n):
                rs = slice(rc * P, (rc + 1) * P)
                m_t = m_ts[rc]
                oh = [
                    outp.tile([P, sw], out_dt, name=f"oq{q}", tag=f"oq{q}")
                    for q in range(2)
                ]
                xpieces = xs[rc]
                last = rc == rc_n - 1
                fine = last  # fine-grained evictions only in the final chunk
                w0 = wp.tile([P, xw], mm1_dt, name="w_t", tag="w_t")
                nc.gpsimd.tensor_mul(w0[:], xpieces[0][:], m_t[:, 0:xw])
                w1 = wp.tile([P, xw], mm1_dt, name="w_t2", tag="w_t")
                if last:
                    # split the final multiply across engines: shortest
                    # path from the last X piece to the last matmul
                    nc.vector.tensor_mul(
                        w1[:, 0:qw], xpieces[1][:, 0:qw], m_t[:, xw : xw + qw]
                    )
                    nc.gpsimd.tensor_mul(
                        w1[:, qw:xw], xpieces[1][:, qw:xw], m_t[:, xw + qw : 2 * xw]
                    )
                else:
                    nc.gpsimd.tensor_mul(w1[:], xpieces[1][:], m_t[:, xw : 2 * xw])
                wref = [(w0, 0), (w0, qw), (w1, 0), (w1, qw)]
                for p in range(4):
                    ps1 = ps1p.tile([P, qw], f32)
                    w_t, wo = wref[p]
                    for t in range(2 * grp):
                        nc.tensor.matmul(
                            ps1[:, t * P : (t + 1) * P],
                            lhsT=w_t[:, wo + t * P : wo + (t + 1) * P],
                            rhs=bt_t[:, rc * P : (rc + 1) * P],
                            start=True,
                            stop=True,
                        )
                    o1 = o1p.tile([P, qw], mm2_dt)
                    if fine:
                        # 512-wide halves on two engines: halves the
                        # mm1->mm2 latency in the drain chunks
                        nc.scalar.copy(o1[:, 0:gw], ps1[:, 0:gw])
                        nc.vector.tensor_copy(o1[:, gw:qw], ps1[:, gw:qw])
                    elif p % 2 == 0:
                        nc.scalar.copy(o1[:], ps1[:])
                    else:
                        nc.vector.tensor_copy(o1[:], ps1[:])
                    ps2 = ps2p.tile([P, qw], f32)
                    for t in range(2 * grp):
                        c = p * 2 * grp + t
                        nc.tensor.matmul(
                            ps2[:, t * P : (t + 1) * P],
                            lhsT=o1[:, t * P : (t + 1) * P],
                            rhs=a_t[:, c * P : (c + 1) * P],
                            start=True,
                            stop=True,
                        )
                    j = p // 2
                    off = (p % 2) * qw
                    if fine:
                        nc.scalar.copy(oh[j][:, off : off + gw], ps2[:, 0:gw])
                        nc.vector.tensor_copy(
                            oh[j][:, off + gw : off + qw], ps2[:, gw:qw]
                        )
                    elif p % 2 == 0:
                        nc.vector.tensor_copy(oh[j][:, off : off + qw], ps2[:])
                    else:
                        nc.scalar.copy(oh[j][:, off : off + qw], ps2[:])
                    if p % 2 == 1:
                        seng = nc.gpsimd if rc in (0, 2) else nc.sync
                        seng.dma_start(o_d[rs, j * sw : (j + 1) * sw], oh[j][:])
    return nc


def host_prep(c_0, c_1, permutations_0, permutations_1, d):
    """Build the block-diagonal mix matrices.

    Returns bt_all [d//128, 128, 128] (chunk, m_local, j_local) and
    amat [128, d] (c_local, chunk*128 + k_local)."""
    k = np.arange(d)
    p0 = np.asarray(permutations_0)
    p1 = np.asarray(permutations_1)
    c0 = np.asarray(c_0, dtype=np.float32)
    c1 = np.asarray(c_1, dtype=np.float32)
    cch = d // P

    bt = np.zeros((d, BLOCK), np.float32)  # [j, m_local]
    for p in range(p0.shape[0]):
        np.add.at(bt, (k, p0[p] % BLOCK), c0[p])
    b4 = bt.reshape(cch, 2, BLOCK, BLOCK)  # [chunk, half, j_loc, m_loc]
    bt_all = np.zeros((cch, P, P), np.float32)
    bt_all[:, :BLOCK, :BLOCK] = b4[:, 0].transpose(0, 2, 1)
    bt_all[:, BLOCK:, BLOCK:] = b4[:, 1].transpose(0, 2, 1)

    a1 = np.zeros((d, BLOCK), np.float32)  # [k, c_local]
    for p in range(p1.shape[0]):
        np.add.at(a1, (k, p1[p] % BLOCK), c1[p])
    a4 = a1.reshape(cch, 2, BLOCK, BLOCK)  # [chunk, half, k_loc, c_loc]
    a_all = np.zeros((cch, P, P), np.float32)
    a_all[:, :BLOCK, :BLOCK] = a4[:, 0].transpose(0, 2, 1)
    a_all[:, BLOCK:, BLOCK:] = a4[:, 1].transpose(0, 2, 1)
    amat = np.ascontiguousarray(a_all.transpose(1, 0, 2).reshape(P, d))
    return bt_all, amat


def _numpy_fallback(X, c_0, c_1, mask, p0, p1):
    W = np.asarray(X, np.float32) * np.asarray(mask)
    W = np.einsum("ipk,pk->ik", W[:, p1], np.asarray(c_1, np.float32))
    W = np.einsum("pjk,pj->jk", W[p0, :], np.asarray(c_0, np.float32))
    return W.astype(np.float32)


def kernel(X, c_0, c_1, mask, permutations_0, permutations_1):
    X = np.asarray(X)
    mask = np.asarray(mask)
    p0 = np.asarray(permutations_0)
    p1 = np.asarray(permutations_1)

    d = X.shape[1]
    k = np.arange(d)
    block_local = (
        X.shape == (D, D)
        and p0.shape == (NP, D)
        and p1.shape == (NP, D)
        and (p0 // BLOCK == k // BLOCK).all()
        and (p1 // BLOCK == k // BLOCK).all()
    )
    if not block_local:
        return _numpy_fallback(X, c_0, c_1, mask, p0, p1)

    from concourse.bass_utils import run_bass_kernel_spmd

    rows = D // NCORES
    cfg = dict(CONFIG)
    key = ("nc", cfg["mm1"], cfg["mm2"], cfg["mask"], cfg["out"], cfg["x"])
    if key not in _CACHE:
        _CACHE[key] = build_bass(rows, D, **cfg)
    nc = _CACHE[key]

    def _mmdt(which):
        if cfg[which] == "bf16":
            import ml_dtypes

            return ml_dtypes.bfloat16
        if cfg[which] == "fp16":
            return np.float16
        if cfg[which] == "fp8":
            import ml_dtypes

            return ml_dtypes.float8_e4m3fn
        return np.float32

    bt_all, amat = host_prep(c_0, c_1, p0, p1, D)
    amat = np.ascontiguousarray(amat.astype(_mmdt("mm2")))
    rc_n = rows // P
    xf = np.ascontiguousarray(X.astype(_mmdt("x"), copy=False))
    mu = np.ascontiguousarray(mask.astype(_mmdt("mask") if cfg["mask"] != "u8" else np.uint8))

    in_maps = []
    for i in range(NCORES):
        rs = slice(i * rows, (i + 1) * rows)
        bt_core = np.ascontiguousarray(
            bt_all[i * rc_n : (i + 1) * rc_n]
            .transpose(1, 0, 2)
            .reshape(P, rc_n * P)
            .astype(_mmdt("mm1"))
        )
        in_maps.append(
            {
                "x": xf[rs],
                "m": mu[rs],
                "bt": bt_core,
                "amat": amat,
            }
        )

    res = run_bass_kernel_spmd(nc, in_maps, list(range(NCORES)), trace=PROFILE)
    LAST["res"] = res
    out = np.concatenate([res.results[i]["out"] for i in range(NCORES)], axis=0)
    return out.astype(np.float32)



# revision 10
# speedup vs baseline: 1.0807x; 1.0351x over previous
"""Trainium2 Bass kernel for nn_PermutedSparseWeight.

Math: out = P0-mix( P1-mix( X*mask ) ) where both mixes are weighted sums
over 8 block-local (64-wide) permutations.  Because every permutation maps
indices within their own 64-block, the whole computation factors into
per-block matrix sandwiches:

    out[block a, block b] = B_a @ (X*mask)[a, b] @ A_b

with B_a[j, m] = sum_p c0[p, j]·[perm0[p, j] == m]   (row mix)
and  A_b[c, k] = sum_p c1[p, k]·[perm1[p, k] == c]   (col mix).

The tiny A/B matrices (1 MB each) are assembled on the host from the
c/perm metadata; all heavy data (X 64 MB, mask 16 MB, out 64 MB) is
processed on device.  d_out is sharded 8 ways (512 rows / core, a
multiple of the 64 block size, keeping row mixes core-local); A is
replicated.

On device, per 128-row chunk (2 blocks): an fp16 matmul with the masked
X-chunk as the stationary operand produces the row-mixed chunk directly
in transposed layout (out1T[c, j] = sum_m Wm[m, c]·BT[m, j]), which is
exactly the lhsT layout the column-mix matmul needs — no transposes.
The mask ships as fp8 (float dtype keeps the DVE multiply on its fast
mixed-float path); all 16-bit dtypes stay well inside the 2e-2 gate
(measured rel err 1.4e-3).
"""

import numpy as np

D = 4096
NP = 8
BLOCK = 64
NCORES = 8
P = 128

_CACHE = {}
PROFILE = False  # test-harness switch: capture NTFF profile on the next run
LAST = {}  # test-harness: BassKernelResults of the most recent run
# matmul dtypes, output dtype and mask handling; see build_bass
CONFIG = {"mm1": "fp16", "mm2": "fp16", "mask": "fp16", "out": "fp16", "x": "fp16"}


_MAXW = 1  # walrus ISA: instructions carry at most one sync wait command


def _patch_tile_drain():
    """The walrus codegen in this environment rejects instructions carrying
    more than _MAXW semaphore waits ("Too many sync wait commands").  Two
    patches, both semantically neutral:
      1. every instruction Tile commits with more waits gets same-engine
         no-op predecessors carrying the overflow waits (engine queues are
         in-order, so the waits still all complete before the instruction);
      2. the TileContext exit drain is split into a chain of drains."""
    import concourse.tile as tile
    import bass_rust
    from concourse.vector_clock import ScopedClock

    if getattr(tile.TileContext, "_drain_patched", False):
        return

    def _split_waits(self, inst):
        si = inst.sync_info
        waits = list(si.on_wait or []) if si else []
        if len(waits) <= _MAXW:
            return
        keep = waits[-_MAXW:]
        extra = waits[: -_MAXW]
        for i in range(0, len(extra), _MAXW):
            nop = bass_rust.InstNoOp(name=self.nc.get_next_instruction_name())
            nop.engine = inst.engine
            nop.sync_info = bass_rust.SyncInfo(
                on_wait=extra[i : i + _MAXW], on_update=[]
            )
            self.nc.register_instruction(nop, overwrite=True)
            self.nc.cur_bb.bb.add_instruction(nop)
        inst.sync_info = bass_rust.SyncInfo(
            on_wait=keep, on_update=list(si.on_update or [])
        )

    orig_add = tile.TileContext._add_instruction

    def _add_instruction(self, inst):
        if inst.engine != tile.mybir.EngineType.Unassigned:
            _split_waits(self, inst)
        orig_add(self, inst)

    def _drain_and_barrier(self, tick_clock, wait_clock):
        drain_inst = self.nc.sync.drain()
        wait_clock.add_sem_waits(
            drain_inst.ins, ScopedClock({None: tick_clock.global_clock})
        )
        si = drain_inst.ins.sync_info
        waits = list(si.on_wait or []) if si else []
        if len(waits) > _MAXW:
            drain_inst.ins.sync_info = bass_rust.SyncInfo(
                on_wait=waits[:_MAXW], on_update=list(si.on_update or [])
            )
            for i in range(_MAXW, len(waits), _MAXW):
                d2 = self.nc.sync.drain()
                si2 = d2.ins.sync_info
                upd = list(si2.on_update or []) if si2 else []
                d2.ins.sync_info = bass_rust.SyncInfo(
                    on_wait=waits[i : i + _MAXW], on_update=upd
                )
        self.nc.all_engine_barrier()
        assert self.sems is not None
        popped = self.nc._tile_sem_poison_stack.pop()
        assert popped is self._sem_poison
        self.nc.clear_and_free_semaphores(list(self.sems.allocated().values()))
        self.nc.all_engine_barrier()

    tile.TileContext._add_instruction = _add_instruction
    tile.TileContext._drain_and_barrier = _drain_and_barrier
    tile.TileContext._drain_patched = True


def build_bass(rows, d, mm1="fp16", mm2="fp16", mask="fp8", out="fp16", x="hybrid"):
    """One-core SPMD program: rows x d shard of X/mask -> rows x d of out.

    Dtype strategy (all HW-measured, ns per 128-lane column):
      multiply  f32*fp8: vector 1.2 | fp16*fp8: gpsimd 1.87, vector 4.2
                fp16*fp16: vector 1.9, gpsimd 2.4 | bf16*bf16: vector 2.4
      evict     scalar ACTIVATE ~0.67, vector CAST ~0.8 (PSUM f32->fp16)
    The only fast vector multiply needs an f32 first operand, and gpsimd's
    best is fp16*fp8 - so X ships HYBRID: chunks 0,3 as f32 (multiplied on
    vector - fast start and fast drain), chunks 1,2 as fp16 (multiplied on
    gpsimd), mask fp8 throughout.  13.06 MB/core total vs 15.06 all-f32,
    with the multiply load split 9.8us vector / 15.3us gpsimd.  Precision
    is unchanged: w is fp16 in every path.  (Dead ends, measured: all-fp16
    X starves on the 4.2ns/col vector path; in-DMA accum masking is
    impossible - DGE compute ops are bypass/add only; 16-bit PSUM is
    TRN3-only.)

    DMA-packet economics (measured): >=4KB DRAM rows sustain ~400 GB/s
    aggregate, 2KB ~300.  f32 X pieces 2048-wide = 8KB rows, fp16 = 4KB,
    fp8 masks full-width = 4KB, amat fp16 full-width = 8KB.

    Ring plan: sync HWDGE carries all X in consumption order then the last
    chunk's store; scalar HWDGE carries m0 (split halves for an early
    first multiply), bt, amat, m1-m3; gpsimd SWDGE carries stores c0-c2.
    PSUM drains: all o1 evictions on scalar (fastest), out evictions
    mostly on vector."""
    import concourse.bass as bass
    import concourse.tile as tile
    from concourse import mybir

    _patch_tile_drain()

    f32 = mybir.dt.float32
    fp16 = mybir.dt.float16
    u8 = mybir.dt.uint8
    dmap = {"f32": mybir.dt.float32, "bf16": mybir.dt.bfloat16,
            "fp16": fp16, "fp8": mybir.dt.float8e4}
    mm1_dt = dmap[mm1]
    mm2_dt = dmap[mm2]
    out_dt = dmap[out]
    m_dt = u8 if mask == "u8" else dmap[mask]

    rc_n = rows // P      # row chunks per core (4)
    cch = d // P          # column chunks (32)
    grp = 4               # col chunks per PSUM bank group / 512-col strip
    gw = grp * P          # 512
    qw = 2 * gw           # pair width (1024)
    xw = 2 * qw           # x piece width (2048)
    # which chunks ship f32 (vector multiply) vs fp16 (gpsimd multiply)
    XF32 = [True, False, False, True]
    assert rc_n == 4

    nc = bass.Bass("TRN2", target_bir_lowering=False, debug=False)
    # NOTE: nc.m.queues each declare num_queues=16 subqueue rings; transfers
    # round-robin across the rings to feed the 16 DMA engines.  Shrinking the
    # declaration (to cut the NEFF epilogue's per-subqueue semaphore clears,
    # ~6 us) collapses DMA throughput ~3.5x - do not touch it.
    xa_d = nc.dram_tensor("x32a", [P, d], f32, kind="ExternalInput").ap()
    x16_d = nc.dram_tensor("x16", [2 * P, d], fp16, kind="ExternalInput").ap()
    xb_d = nc.dram_tensor("x32b", [P, d], f32, kind="ExternalInput").ap()
    m_d = nc.dram_tensor("m", [rows, d], m_dt, kind="ExternalInput").ap()
    bt_d = nc.dram_tensor("bt", [P, rc_n * P], mm1_dt, kind="ExternalInput").ap()
    a_d = nc.dram_tensor("amat", [P, d], mm2_dt, kind="ExternalInput").ap()
    o_d = nc.dram_tensor("out", [rows, d], out_dt, kind="ExternalOutput").ap()
    x_srcs = [  # (dram ap, row offset, dtype) per chunk
        (xa_d, 0, f32),
        (x16_d, 0, fp16),
        (x16_d, P, fp16),
        (xb_d, 0, f32),
    ]

    with tile.TileContext(nc) as tc:
        with (
            tc.tile_pool(name="const", bufs=1) as constp,
            tc.tile_pool(name="xin", bufs=5) as xp,
            tc.tile_pool(name="min", bufs=4) as mp,
            tc.tile_pool(name="wq", bufs=4) as wp,
            tc.tile_pool(name="o1", bufs=3) as o1p,
            tc.tile_pool(name="osb", bufs=4) as outp,
            tc.tile_pool(name="ps1", bufs=2, space="PSUM") as ps1p,
            tc.tile_pool(name="ps2", bufs=2, space="PSUM") as ps2p,
        ):
            # Warm the ACT table during the DMA ramp: the first real
            # ACTIVATE otherwise pays a ~1.3us ACT_TABLE_LOAD on the
            # chunk-0 critical path.
            warm = constp.tile([P, 2], f32, name="actwarm")
            nc.gpsimd.memset(warm[:, 0:1], 0.0)
            nc.scalar.copy(warm[:, 1:2], warm[:, 0:1])

            # sync ring: all X pieces in consumption order.
            xs = [None] * rc_n
            for rc in range(rc_n):
                src, ro, xdt = x_srcs[rc]
                tag = "xf_t" if XF32[rc] else "xh_t"
                ps_ = []
                for j in range(d // xw):
                    x_t = xp.tile([P, xw], xdt, name=f"x{rc}_{j}", tag=tag)
                    nc.sync.dma_start(
                        x_t[:], src[ro : ro + P, j * xw : (j + 1) * xw]
                    )
                    ps_.append(x_t)
                xs[rc] = ps_

            # scalar ring in need order: m0 first half, bt, m0 second half,
            # amat, then the remaining masks.
            m_ts = [None] * rc_n
            m0 = mp.tile([P, d], m_dt, name="m0", tag="m_t")
            h = d // 2
            nc.scalar.dma_start(m0[:, 0:h], m_d[0:P, 0:h])
            bt_t = constp.tile([P, rc_n * P], mm1_dt)
            nc.scalar.dma_start(bt_t[:], bt_d[:])
            nc.scalar.dma_start(m0[:, h:d], m_d[0:P, h:d])
            m_ts[0] = m0
            a_t = constp.tile([P, d], mm2_dt, name="amat", tag="amat")
            nc.scalar.dma_start(a_t[:], a_d[:])
            for rc in range(1, rc_n):
                m_t = mp.tile([P, d], m_dt, name=f"m{rc}", tag="m_t")
                nc.scalar.dma_start(m_t[:], m_d[rc * P : (rc + 1) * P, :])
                m_ts[rc] = m_t

            # ---- compute ----
            # Per chunk: two 2048-wide mask-multiplies (vector for f32
            # chunks, gpsimd for fp16 chunks), then 4 "pairs" of 1024
            # cols: 8 mm1 matmuls into a 2-bank PSUM tile, a 1024-wide o1
            # eviction (scalar), 8 mm2 matmuls, a 1024-wide eviction into
            # the store buffer (vector, sharing with scalar).  Stores go
            # 2048-wide on gpsimd (c0-c2) and sync (c3).
            sw = 2 * qw  # store piece width (2048)
            for rc in range(rc_n):
                rs = slice(rc * P, (rc + 1) * P)
                m_t = m_ts[rc]
                oh = [
                    outp.tile([P, sw], out_dt, name=f"oq{q}", tag=f"oq{q}")
                    for q in range(2)
                ]
                xpieces = xs[rc]
                meng = nc.vector if XF32[rc] else nc.gpsimd
                last = rc == rc_n - 1
                fine = last  # fine-grained evictions only in the final chunk
                w0 = wp.tile([P, xw], mm1_dt, name="w_t", tag="w_t")
                meng.tensor_mul(w0[:], xpieces[0][:], m_t[:, 0:xw])
                w1 = wp.tile([P, xw], mm1_dt, name="w_t2", tag="w_t")
                if last:
                    # two 1024-wide halves: shortest path from the last X
                    # piece to the last matmul
                    meng.tensor_mul(
                        w1[:, 0:qw], xpieces[1][:, 0:qw], m_t[:, xw : xw + qw]
                    )
                    meng.tensor_mul(
                        w1[:, qw:xw], xpieces[1][:, qw:xw], m_t[:, xw + qw : 2 * xw]
                    )
                else:
                    meng.tensor_mul(w1[:], xpieces[1][:], m_t[:, xw : 2 * xw])
                wref = [(w0, 0), (w0, qw), (w1, 0), (w1, qw)]
                for p in range(4):
                    ps1 = ps1p.tile([P, qw], f32)
                    w_t, wo = wref[p]
                    for t in range(2 * grp):
                        nc.tensor.matmul(
                            ps1[:, t * P : (t + 1) * P],
                            lhsT=w_t[:, wo + t * P : wo + (t + 1) * P],
                            rhs=bt_t[:, rc * P : (rc + 1) * P],
                            start=True,
                            stop=True,
                        )
                    o1 = o1p.tile([P, qw], mm2_dt)
                    if fine:
                        # 512-wide halves on two engines: halves the
                        # mm1->mm2 latency in the drain chunks
                        nc.scalar.copy(o1[:, 0:gw], ps1[:, 0:gw])
                        nc.vector.tensor_copy(o1[:, gw:qw], ps1[:, gw:qw])
                    else:
                        nc.scalar.copy(o1[:], ps1[:])
                    ps2 = ps2p.tile([P, qw], f32)
                    for t in range(2 * grp):
                        c = p * 2 * grp + t
                        nc.tensor.matmul(
                            ps2[:, t * P : (t + 1) * P],
                            lhsT=o1[:, t * P : (t + 1) * P],
                            rhs=a_t[:, c * P : (c + 1) * P],
                            start=True,
                            stop=True,
                        )
                    j = p // 2
                    off = (p % 2) * qw
                    if fine:
                        nc.scalar.copy(oh[j][:, off : off + gw], ps2[:, 0:gw])
                        nc.vector.tensor_copy(
                            oh[j][:, off + gw : off + qw], ps2[:, gw:qw]
                        )
                    elif p == 3:
                        nc.scalar.copy(oh[j][:, off : off + qw], ps2[:])
                    else:
                        nc.vector.tensor_copy(oh[j][:, off : off + qw], ps2[:])
                    if p % 2 == 1:
                        seng = nc.sync if last else nc.gpsimd
                        seng.dma_start(o_d[rs, j * sw : (j + 1) * sw], oh[j][:])
    return nc


def host_prep(c_0, c_1, permutations_0, permutations_1, d):
    """Build the block-diagonal mix matrices.

    Returns bt_all [d//128, 128, 128] (chunk, m_local, j_local) and
    amat [128, d] (c_local, chunk*128 + k_local)."""
    k = np.arange(d)
    p0 = np.asarray(permutations_0)
    p1 = np.asarray(permutations_1)
    c0 = np.asarray(c_0, dtype=np.float32)
    c1 = np.asarray(c_1, dtype=np.float32)
    cch = d // P

    bt = np.zeros((d, BLOCK), np.float32)  # [j, m_local]
    for p in range(p0.shape[0]):
        np.add.at(bt, (k, p0[p] % BLOCK), c0[p])
    b4 = bt.reshape(cch, 2, BLOCK, BLOCK)  # [chunk, half, j_loc, m_loc]
    bt_all = np.zeros((cch, P, P), np.float32)
    bt_all[:, :BLOCK, :BLOCK] = b4[:, 0].transpose(0, 2, 1)
    bt_all[:, BLOCK:, BLOCK:] = b4[:, 1].transpose(0, 2, 1)

    a1 = np.zeros((d, BLOCK), np.float32)  # [k, c_local]
    for p in range(p1.shape[0]):
        np.add.at(a1, (k, p1[p] % BLOCK), c1[p])
    a4 = a1.reshape(cch, 2, BLOCK, BLOCK)  # [chunk, half, k_loc, c_loc]
    a_all = np.zeros((cch, P, P), np.float32)
    a_all[:, :BLOCK, :BLOCK] = a4[:, 0].transpose(0, 2, 1)
    a_all[:, BLOCK:, BLOCK:] = a4[:, 1].transpose(0, 2, 1)
    amat = np.ascontiguousarray(a_all.transpose(1, 0, 2).reshape(P, d))
    return bt_all, amat


def _numpy_fallback(X, c_0, c_1, mask, p0, p1):
    W = np.asarray(X, np.float32) * np.asarray(mask)
    W = np.einsum("ipk,pk->ik", W[:, p1], np.asarray(c_1, np.float32))
    W = np.einsum("pjk,pj->jk", W[p0, :], np.asarray(c_0, np.float32))
    return W.astype(np.float32)


def kernel(X, c_0, c_1, mask, permutations_0, permutations_1):
    X = np.asarray(X)
    mask = np.asarray(mask)
    p0 = np.asarray(permutations_0)
    p1 = np.asarray(permutations_1)

    d = X.shape[1]
    k = np.arange(d)
    block_local = (
        X.shape == (D, D)
        and p0.shape == (NP, D)
        and p1.shape == (NP, D)
        and (p0 // BLOCK == k // BLOCK).all()
        and (p1 // BLOCK == k // BLOCK).all()
    )
    if not block_local:
        return _numpy_fallback(X, c_0, c_1, mask, p0, p1)

    from concourse.bass_utils import run_bass_kernel_spmd

    rows = D // NCORES
    cfg = dict(CONFIG)
    key = ("nc", cfg["mm1"], cfg["mm2"], cfg["mask"], cfg["out"], cfg["x"])
    if key not in _CACHE:
        _CACHE[key] = build_bass(rows, D, **cfg)
    nc = _CACHE[key]

    def _mmdt(which):
        if cfg[which] == "bf16":
            import ml_dtypes

            return ml_dtypes.bfloat16
        if cfg[which] == "fp16":
            return np.float16
        if cfg[which] == "fp8":
            import ml_dtypes

            return ml_dtypes.float8_e4m3fn
        return np.float32

    bt_all, amat = host_prep(c_0, c_1, p0, p1, D)
    amat = np.ascontiguousarray(amat.astype(_mmdt("mm2")))
    rc_n = rows // P
    # hybrid X shipping: per core, chunk 0 and 3 as f32 (vector multiply),
    # chunks 1-2 as fp16 (gpsimd multiply); see build_bass docstring
    xf32 = np.ascontiguousarray(X.astype(np.float32, copy=False))
    xf16 = X.astype(np.float16)
    mu = np.ascontiguousarray(mask.astype(_mmdt("mask") if cfg["mask"] != "u8" else np.uint8))

    in_maps = []
    for i in range(NCORES):
        rs = slice(i * rows, (i + 1) * rows)
        bt_core = np.ascontiguousarray(
            bt_all[i * rc_n : (i + 1) * rc_n]
            .transpose(1, 0, 2)
            .reshape(P, rc_n * P)
            .astype(_mmdt("mm1"))
        )
        r0 = i * rows
        in_maps.append(
            {
                "x32a": np.ascontiguousarray(xf32[r0 : r0 + P]),
                "x16": np.ascontiguousarray(xf16[r0 + P : r0 + 3 * P]),
                "x32b": np.ascontiguousarray(xf32[r0 + 3 * P : r0 + 4 * P]),
                "m": mu[rs],
                "bt": bt_core,
                "amat": amat,
            }
        )

    res = run_bass_kernel_spmd(nc, in_maps, list(range(NCORES)), trace=PROFILE)
    LAST["res"] = res
    out = np.concatenate([res.results[i]["out"] for i in range(NCORES)], axis=0)
    return out.astype(np.float32)

